# revision 41
# baseline (speedup 1.0000x reference)
"""DGCNN hypergraph kernel for Trainium2 (Bass/Tile), 8-core SPMD.

Strategy (per the data-parallel sharding hint): 128 disjoint hypergraphs are
sharded 16-per-core across 8 NeuronCores. All message passing is graph-local.

The incidence matrices A (node->edge counts), their transposes At, and the
hyperedge sizes are pure functions of the integer incidence input, so they are
built on the host (exact small-integer fp16) and DMAed in; the device runs only
the floating-point pipeline.

Per-core pipeline (16 graphs, processed as 2 pairs-of-groups; the two groups of
a pair are interleaved sub-stage by sub-stage so their dependency chains
overlap on all engines with only bufs=2 PSUM pools):
  - 4 conv layers x 2 directions per group: linear (fp32 matmul, block-diag
    weights for 4-graph batching), PE transpose to node-major, fp16 hi/lo pair
    split, and aggregation as col-tiled fp16 matmuls against A / At accumulated
    in PSUM (2-pass hi/lo gives ~fp32 accuracy), then bias/degree-scale + tanh.
    Aggregation matmuls are issued graph-interleaved so the four 32-column PE
    tile chains stream concurrently.
  - Sort-pooling per pair: top-30 per graph via max8/max_index/match_replace
    rounds (tie behavior matches jax stable top_k), gather via ap_gather.
  - Conv tower + dense layer via small fp32 matmuls, relu, output assembly.
"""

import numpy as np
from contextlib import ExitStack

import concourse.bass as bass
import concourse.tile as tile
from concourse import bacc, mybir
from concourse.bass_utils import run_bass_kernel_spmd

dt = mybir.dt
ALU = mybir.AluOpType
AF = mybir.ActivationFunctionType
AX = mybir.AxisListType

B = 128          # graphs
NPER = 512       # nodes per graph
EPER = 512       # hyperedges per graph
DEG = 32         # memberships per node
F = 128          # input feature dim
K = 30           # sortpool k
NCORES = 8
GPC = B // NCORES          # 16 graphs per core
NGROUP = GPC // 4          # 4 groups of 4 graphs
NPAIR = NGROUP // 2        # 2 pairs of groups
C1, C2, KW2 = 16, 32, 5
HDEG = float(DEG + 1)      # node hyperdegree + 1 (structural: 33)

# packed-constant free-dim offsets ([128, CW_TOT] fp32)
_OFF_BDE = 0          # 3 x 128
_OFF_BDN = 384        # 4 x 128
_OFF_IDENT = 896      # 128
_OFF_W0 = 1024        # 32
_OFF_CW1 = 1056       # 4 x 16
_OFF_CW2 = 1120       # 5 x 32
_OFF_OW = 1280        # 2 x 11
_OFF_CB1 = 1302       # 1
_OFF_CB2 = 1303       # 1
_OFF_BEPP = 1304      # 4 x 1
_OFF_BNPP = 1308      # 4 x 1
_OFF_SSUM = 1312      # 4
CW_TOT = 1316

_CACHE = {}


def _pad32(w):
    out = np.zeros((32, 32), np.float32)
    out[: w.shape[0], : w.shape[1]] = w
    return out


def _blockdiag4(w):
    out = np.zeros((128, 128), np.float32)
    for g in range(4):
        out[32 * g : 32 * g + 32, 32 * g : 32 * g + 32] = w
    return out


def _build_program():
    nc = bacc.Bacc("TRN2", target_bir_lowering=False, debug=False,
                   num_devices=NCORES)

    # ---- DRAM I/O ----
    CONST = nc.dram_tensor("constpk", [128, CW_TOT], dt.float32, kind="ExternalInput").ap()
    OUTB = nc.dram_tensor("outb", [4, 8], dt.float32, kind="ExternalInput").ap()
    APK = nc.dram_tensor("apk", [NGROUP, 128, 4, 4, 512], dt.float16, kind="ExternalInput").ap()
    ATPK = nc.dram_tensor("atpk", [NGROUP, 128, 4, 4, 512], dt.float16, kind="ExternalInput").ap()
    FPK = nc.dram_tensor("fpk", [NGROUP, 128, 5, 512], dt.float32, kind="ExternalInput").ap()
    OUT = nc.dram_tensor("out", [GPC, 2], dt.float32, kind="ExternalOutput").ap()
    IDXD = nc.dram_tensor("idxd", [NPAIR, 2, 128, 2], dt.int16, kind="Internal").ap()

    with tile.TileContext(nc) as tc, ExitStack() as ctx:
        cpool = ctx.enter_context(tc.tile_pool(name="consts", bufs=1))
        gpool = ctx.enter_context(tc.tile_pool(name="graph", bufs=1))
        apool = ctx.enter_context(tc.tile_pool(name="amat", bufs=2))
        atpoolA = ctx.enter_context(tc.tile_pool(name="atmatA", bufs=2))
        atpoolB = ctx.enter_context(tc.tile_pool(name="atmatB", bufs=2))
        hpool = ctx.enter_context(tc.tile_pool(name="acts", bufs=2))
        hcatp = ctx.enter_context(tc.tile_pool(name="hcat", bufs=4))
        tpool = ctx.enter_context(tc.tile_pool(name="tmp", bufs=2))
        t3pool = ctx.enter_context(tc.tile_pool(name="tmp3", bufs=3))
        kpool = ctx.enter_context(tc.tile_pool(name="keys", bufs=1))
        pzn = ctx.enter_context(tc.tile_pool(name="pzn", bufs=2, space="PSUM"))
        pagg = ctx.enter_context(tc.tile_pool(name="pagg", bufs=5, space="PSUM"))
        ps2 = ctx.enter_context(tc.tile_pool(name="ps2", bufs=1, space="PSUM"))
        # bank budget (8): pzn x3 + pagg x3 + ps2 x2 = 8

        ct = cpool.tile([128, CW_TOT], dt.float32, tag="constpk")
        nc.sync.dma_start(ct[:], CONST)
        outb = cpool.tile([4, 8], dt.float32, tag="outb")
        nc.sync.dma_start(outb[:], OUTB)

        w0 = ct[:, _OFF_W0 : _OFF_W0 + 32]
        bde = [ct[:, _OFF_BDE + 128 * l : _OFF_BDE + 128 * l + 128] for l in range(3)]
        bdn = [ct[:, _OFF_BDN + 128 * l : _OFF_BDN + 128 * l + 128] for l in range(4)]
        bepp = [ct[:, _OFF_BEPP + l : _OFF_BEPP + l + 1] for l in range(4)]
        bnpp = [ct[:, _OFF_BNPP + l : _OFF_BNPP + l + 1] for l in range(4)]
        ident = ct[:, _OFF_IDENT : _OFF_IDENT + 128]
        cw1 = [ct[:, _OFF_CW1 + 16 * l : _OFF_CW1 + 16 * l + 16] for l in range(4)]
        cb1 = ct[:, _OFF_CB1 : _OFF_CB1 + 1]
        cw2 = [ct[:, _OFF_CW2 + 32 * d : _OFF_CW2 + 32 * d + 32] for d in range(5)]
        cb2 = ct[:, _OFF_CB2 : _OFF_CB2 + 1]
        ow = [ct[:, _OFF_OW + 11 * o : _OFF_OW + 11 * o + 11] for o in range(2)]
        ssum = ct[:, _OFF_SSUM : _OFF_SSUM + 4]

        # pair P uses partitions 32P..32P+8 (engine ops need 32-aligned bases)
        keysB = kpool.tile([64, 512], dt.float32, tag="keysB")
        Yout = kpool.tile([128, 8], dt.float32, tag="yout")

        def direction(gi, l, side, st):
            """One message-passing direction for group-slot gi.

            The linear transform is computed with the activations as the
            stationary operand (lhsT = hT chunk, rhs = block-diag weights), so
            the result lands in PSUM directly in contraction-major layout
            [node/edge partition, (chunk, graph, feature) columns] — no PE
            transpose or PSUM->SBUF staging copy is needed.
            """
            hT_in = st["hT"]
            zN = pzn.tile([128, 512], dt.float32, tag="zn")
            if side == "E" and l == 0:
                for c in range(4):
                    for g in range(4):
                        nc.tensor.matmul(
                            zN[:, 128 * c + 32 * g : 128 * c + 32 * g + 32],
                            st["fpk"][:, g, 128 * c : 128 * c + 128], w0,
                            start=True, stop=True)
            else:
                src = hT_in[:] if side == "E" else st["heT"][:]
                bdw = bde[l - 1] if side == "E" else bdn[l]
                for c in range(4):
                    nc.tensor.matmul(zN[:, 128 * c : 128 * c + 128],
                                     src[:, 128 * c : 128 * c + 128], bdw,
                                     start=True, stop=True)
            zhi = t3pool.tile([128, 512], dt.float16, tag="zhi")
            nc.scalar.copy(zhi[:], zN[:])
            zlo = t3pool.tile([128, 512], dt.float16, tag="zlo")
            nc.vector.tensor_tensor(zlo[:], zN[:], zhi[:], ALU.subtract)

            # aggregation, graph-interleaved issue: the four per-graph
            # accumulation chains target distinct 32-col PE tiles so adjacent
            # matmuls stream concurrently; per-graph PSUM accumulation order
            # (zhi c0..c3 then zlo c0..c3) matches the reference kernel.
            Am = st["A"] if side == "E" else st["At"]
            agg = pagg.tile([128, 512], dt.float32, tag="agg")
            n = 0
            for zp in (zhi, zlo):
                for c in range(4):
                    for g in range(4):
                        nc.tensor.matmul(
                            agg[32 * g : 32 * g + 32, :],
                            zp[:, 128 * c + 32 * g : 128 * c + 32 * g + 32],
                            Am[:, g, c, :], start=(n == 0), stop=(n == 7),
                            tile_position=(0, 32 * g))
                    n += 1
            if side == "E":
                ue = tpool.tile([128, 512], dt.float32, tag="ue")
                nc.vector.scalar_tensor_tensor(ue[:], agg[:], bepp[l], st["recip"][:],
                                               ALU.add, ALU.mult)
                heT = hpool.tile([128, 512], dt.float32, tag="heT")
                nc.scalar.activation(heT[:], ue[:], AF.Tanh)
                st["heT"] = heT
            else:
                hT = hcatp.tile([128, 512], dt.float32, tag=f"hT{l}")
                nc.scalar.activation(hT[:], agg[:], AF.Tanh, bias=bnpp[l],
                                     scale=1.0 / HDEG)
                st["hT"] = hT
                st["hcat"][l] = hT
                if l == 3:
                    r = st["krow"]
                    krows = hT[:].rearrange("(a b) f -> a b f", b=32)[:, 0, :]
                    nc.sync.dma_start(keysB[r : r + 4, :], krows)

        all_states = []
        for P in range(NPAIR):
            states = []
            # issue fpk/A for both groups before the (later-needed) At tiles
            fpks, Amats = [], []
            for j in range(2):
                G = 2 * P + j
                fpk = gpool.tile([128, 5, 512], dt.float32, tag=f"fpk{j}")
                nc.sync.dma_start(fpk[:], FPK[G])
                fpks.append(fpk)
                A = apool.tile([128, 4, 4, 512], dt.float16, tag=f"A{j}")
                nc.sync.dma_start(A[:], APK[G])
                Amats.append(A)
            Atmats = []
            for j in range(2):
                G = 2 * P + j
                atp = atpoolA if j == 0 else atpoolB
                At = atp.tile([128, 4, 4, 512], dt.float16, tag=f"At{j}")
                nc.sync.dma_start(At[:], ATPK[G])
                Atmats.append(At)
            for j in range(2):
                G = 2 * P + j
                fpk, A, At = fpks[j], Amats[j], Atmats[j]
                st = {
                    "G": G,
                    "krow": 32 * P + 4 * j,
                    "fpk": fpk,
                    "A": A[:], "At": At[:],
                    "hcat": [None] * 4,
                    "hT": None, "heT": None,
                }
                recip = hpool.tile([128, 512], dt.float32, tag="recip")
                nc.vector.reciprocal(recip[:], fpk[:, 4, :])
                st["recip"] = recip
                states.append(st)

            for l in range(4):
                for side in ("E", "N"):
                    for j in range(2):
                        direction(j, l, side, states[j])

            # ---- per-pair top-k (partitions 32P..32P+8 of the key tiles) ----
            r0 = 32 * P
            kw = keysB   # destructive top-k: keys are not needed afterwards
            idxu = kpool.tile([64, 32], dt.uint32, tag="idxu")
            for r in range(4):
                m8 = kpool.tile([64, 8], dt.float32, tag="m8")
                nc.vector.max(m8[r0 : r0 + 8, :], kw[r0 : r0 + 8, :])
                nc.vector.max_index(idxu[r0 : r0 + 8, 8 * r : 8 * r + 8],
                                    m8[r0 : r0 + 8, :], kw[r0 : r0 + 8, :])
                nc.vector.match_replace(kw[r0 : r0 + 8, :], m8[r0 : r0 + 8, :],
                                        kw[r0 : r0 + 8, :], -1e30)
            idx16 = kpool.tile([64, 32], dt.int16, tag="idx16")
            nc.vector.tensor_copy(idx16[r0 : r0 + 8, :], idxu[r0 : r0 + 8, :])
            # pre-wrap into ap_gather layout: row m becomes 2x-replicated
            # (idx[0], idx[16], idx[1], idx[17], ...) so a plain partition-
            # scatter DMA lands idx i at partition i%16, col i//16
            idx16i = kpool.tile([64, 64], dt.int16, tag="idx16i")
            wsrc = idx16[r0 : r0 + 8, :].rearrange("m (t lo) -> m lo t", lo=16) \
                .unsqueeze(1).broadcast_to([8, 2, 16, 2])
            wdst = idx16i[r0 : r0 + 8, :].rearrange("m (s lo t) -> m s lo t", s=2, t=2)
            nc.vector.tensor_copy(wdst, wsrc)
            for j in range(2):
                nc.sync.dma_start(
                    IDXD[P, j].rearrange("(m p) t -> m (p t)", m=4),
                    idx16i[r0 + 4 * j : r0 + 4 * j + 4, :])

            # ---- pooled gather + conv tower per group of this pair ----
            for j in range(2):
                G = 2 * P + j
                tiles = states[j]["hcat"]
                idxw = tpool.tile([128, 2], dt.int16, tag="idxw")
                nc.sync.dma_start(idxw[:], IDXD[P, j])

                pgs = []
                for l in range(4):
                    pg = tpool.tile([128, 32], dt.float32, tag=f"pg{l}")
                    nc.gpsimd.ap_gather(pg[:], tiles[l][:].unsqueeze(2), idxw[:],
                                        channels=128, num_elems=512, d=1, num_idxs=32)
                    pgs.append(pg)

                y1 = ps2.tile([128, 30], dt.float32, tag="small")
                for g in range(4):
                    for l in range(4):
                        nc.tensor.matmul(y1[32 * g : 32 * g + 16, :],
                                         cw1[l][32 * g : 32 * g + 32, :],
                                         pgs[l][32 * g : 32 * g + 32, 0:30],
                                         start=(l == 0), stop=(l == 3),
                                         tile_position=(32 * g, 32 * g))
                y1r = tpool.tile([128, 30], dt.float32, tag="y1r")
                nc.scalar.activation(y1r[:], y1[:], AF.Relu, bias=cb1)
                y1p = tpool.tile([128, 15], dt.float32, tag="y1p")
                nc.vector.tensor_tensor(
                    y1p[:], y1r[:].rearrange("p (t two) -> p t two", two=2)[:, :, 0],
                    y1r[:].rearrange("p (t two) -> p t two", two=2)[:, :, 1], ALU.max)

                y2 = ps2.tile([128, 11], dt.float32, tag="small")
                for g in range(4):
                    for d in range(5):
                        nc.tensor.matmul(y2[32 * g : 32 * g + 32, :],
                                         cw2[d][32 * g : 32 * g + 32, :],
                                         y1p[32 * g : 32 * g + 32, d : d + 11],
                                         start=(d == 0), stop=(d == 4),
                                         tile_position=(32 * g, 32 * g))
                y2r = tpool.tile([128, 11], dt.float32, tag="y2r")
                nc.scalar.activation(y2r[:], y2[:], AF.Relu, bias=cb2)
                for o in range(2):
                    t_o = tpool.tile([128, 11], dt.float32, tag="t_o")
                    nc.vector.tensor_tensor(t_o[:], y2r[:], ow[o], ALU.mult)
                    nc.vector.tensor_reduce(Yout[:, 2 * G + o : 2 * G + o + 1],
                                            t_o[:], AX.X, ALU.add)

        # ---------- final dense + relu + output ----------
        pout = ps2.tile([4, 8], dt.float32, tag="small")
        nc.tensor.matmul(pout[:], ssum, Yout[:], start=True, stop=True)
        ob = kpool.tile([4, 8], dt.float32, tag="ob")
        nc.vector.tensor_tensor(ob[:], pout[:], outb[:], ALU.add)
        orl = kpool.tile([4, 8], dt.float32, tag="orl")
        nc.scalar.activation(orl[:], ob[:], AF.Relu)
        nc.sync.dma_start(OUT.rearrange("(G g) o -> g G o", g=4), orl[:])

    nc.compile()
    return nc


def _make_consts(inputs):
    ws = [inputs[f"w{i}"].astype(np.float32) for i in range(8)]
    bs = [inputs[f"b{i}"].astype(np.float32) for i in range(8)]
    wE = [ws[0], _pad32(ws[2]), _pad32(ws[4]), _pad32(ws[6])]
    wN = [_pad32(ws[1]), _pad32(ws[3]), _pad32(ws[5]), _pad32(ws[7])]
    bE = [bs[0], bs[2], bs[4], np.pad(bs[6], (0, 31))]
    bN = [bs[1], bs[3], bs[5], np.pad(bs[7], (0, 31))]

    cpk = np.zeros((128, CW_TOT), np.float32)
    for l in range(1, 4):
        cpk[:, _OFF_BDE + 128 * (l - 1) : _OFF_BDE + 128 * l] = _blockdiag4(wE[l])
    for l in range(4):
        cpk[:, _OFF_BDN + 128 * l : _OFF_BDN + 128 * (l + 1)] = _blockdiag4(wN[l])
    cpk[:, _OFF_IDENT : _OFF_IDENT + 128] = np.eye(128, dtype=np.float32)
    cpk[:, _OFF_W0 : _OFF_W0 + 32] = ws[0]
    for l in range(4):
        cpk[:, _OFF_BEPP + l] = np.tile(bE[l], 4)
        cpk[:, _OFF_BNPP + l] = np.tile(bN[l], 4) / HDEG

    c1w = inputs["conv1_w"].astype(np.float32).reshape(C1, 97)    # [16, 97]
    for l in range(4):
        blk = np.zeros((32, 16), np.float32)
        if l < 3:
            blk = c1w[:, 32 * l : 32 * l + 32].T
        else:
            blk[0, :] = c1w[:, 96]
        for g in range(4):
            cpk[32 * g : 32 * g + 32, _OFF_CW1 + 16 * l : _OFF_CW1 + 16 * (l + 1)] = blk
    for g in range(4):
        cpk[32 * g : 32 * g + 16, _OFF_CB1] = inputs["conv1_b"]
    c2w = inputs["conv2_w"].astype(np.float32)                    # [32, 16, 5]
    for d in range(5):
        for g in range(4):
            cpk[32 * g : 32 * g + 16, _OFF_CW2 + 32 * d : _OFF_CW2 + 32 * (d + 1)] = c2w[:, :, d].T
    for g in range(4):
        cpk[32 * g : 32 * g + 32, _OFF_CB2] = inputs["conv2_b"]
    oww = inputs["out_w"].astype(np.float32)                      # [352, 2]
    for o in range(2):
        for g in range(4):
            cpk[32 * g : 32 * g + 32, _OFF_OW + 11 * o : _OFF_OW + 11 * (o + 1)] = \
                oww[:, o].reshape(C2, 11)
    for j in range(4):
        cpk[32 * j : 32 * j + 32, _OFF_SSUM + j] = 1.0

    outb = np.tile(inputs["out_b"].astype(np.float32), (4, 4))    # [4, 8]
    return {"constpk": cpk, "outb": outb}


def _build_incidence(inputs):
    """Host-side A / At / hyperedge-size construction (exact small ints)."""
    einc_g = np.asarray(inputs["inc_edge"]).reshape(B, NPER, DEG).astype(np.int64)
    base = (np.arange(B, dtype=np.int64) * EPER)[:, None, None]
    loc = einc_g - base                                           # [B, 512, 32] local
    A_all = np.empty((B, 128, 4, EPER), np.float16)
    At_all = np.empty((B, 128, 4, NPER), np.float16)
    hs_all = np.empty((B, EPER), np.float32)
    rowbase = (np.arange(NPER, dtype=np.int64) * EPER)[:, None]
    for g in range(B):
        flat = (rowbase + loc[g]).ravel()
        cnt = np.bincount(flat, minlength=NPER * EPER).reshape(NPER, EPER)
        c16 = cnt.astype(np.float16)                              # [n, e]
        A_all[g] = c16.reshape(4, 128, EPER).transpose(1, 0, 2)
        At_all[g] = np.ascontiguousarray(c16.T).reshape(4, 128, NPER).transpose(1, 0, 2)
        hs_all[g] = cnt.sum(axis=0).astype(np.float32) + 1.0
    return A_all, At_all, hs_all


def make_in_maps(inputs):
    consts = _make_consts(inputs)
    nf = np.asarray(inputs["node_feat"]).astype(np.float32).reshape(B, NPER, F)
    nfT = nf.transpose(0, 2, 1)                                   # [B, 128f, 512n]
    A_all, At_all, hs_all = _build_incidence(inputs)
    in_maps = []
    for c in range(NCORES):
        m = dict(consts)
        sl = slice(c * GPC, (c + 1) * GPC)
        # apk/atpk: [NGROUP, 128, 4g, 4c, 512]
        m["apk"] = np.ascontiguousarray(
            A_all[sl].reshape(NGROUP, 4, 128, 4, EPER).transpose(0, 2, 1, 3, 4))
        m["atpk"] = np.ascontiguousarray(
            At_all[sl].reshape(NGROUP, 4, 128, 4, NPER).transpose(0, 2, 1, 3, 4))
        # fpk: [NGROUP, 128, 5, 512] = 4 transposed-feature graphs + hsize row
        fpk = np.empty((NGROUP, 128, 5, 512), np.float32)
        nfT_c = nfT[sl].reshape(NGROUP, 4, 128, NPER)
        for G in range(NGROUP):
            for g in range(4):
                fpk[G, :, g, :] = nfT_c[G, g]
        hs_core = hs_all[sl].reshape(NGROUP, 4, EPER)
        fpk[:, :, 4, :] = np.repeat(hs_core[:, :, None, :], 32, axis=2).reshape(
            NGROUP, 128, EPER)
        m["fpk"] = np.ascontiguousarray(fpk)
        in_maps.append(m)
    return in_maps


def get_program():
    if "nc" not in _CACHE:
        _CACHE["nc"] = _build_program()
    return _CACHE["nc"]


def kernel(**inputs):
    nc = get_program()
    in_maps = make_in_maps(inputs)
    res = run_bass_kernel_spmd(nc, in_maps, core_ids=list(range(NCORES)))
    out = np.concatenate([res.results[c]["out"] for c in range(NCORES)], axis=0)
    return out.astype(np.float32)


# revision 42
# speedup vs baseline: 9846.5869x; 9846.5869x over previous
"""DGCNN hypergraph kernel for Trainium2 (Bass/Tile), 8-core SPMD.

Strategy (per the data-parallel sharding hint): 128 disjoint hypergraphs are
sharded 16-per-core across 8 NeuronCores. All message passing is graph-local.

The incidence matrices A (node->edge counts), their transposes At, and the
hyperedge sizes are pure functions of the integer incidence input, so they are
built on the host (exact small-integer fp16) and DMAed in; the device runs only
the floating-point pipeline.

Per-core pipeline (16 graphs, processed as 2 pairs-of-groups; the two groups of
a pair are interleaved sub-stage by sub-stage so their dependency chains
overlap on all engines with only bufs=2 PSUM pools):
  - 4 conv layers x 2 directions per group: linear (fp32 matmul, block-diag
    weights for 4-graph batching), PE transpose to node-major, fp16 hi/lo pair
    split, and aggregation as col-tiled fp16 matmuls against A / At accumulated
    in PSUM (2-pass hi/lo gives ~fp32 accuracy), then bias/degree-scale + tanh.
    Aggregation matmuls are issued graph-interleaved so the four 32-column PE
    tile chains stream concurrently.
  - Sort-pooling per pair: top-30 per graph via max8/max_index/match_replace
    rounds (tie behavior matches jax stable top_k), gather via ap_gather.
  - Conv tower + dense layer via small fp32 matmuls, relu, output assembly.
"""

import numpy as np
from contextlib import ExitStack

import concourse.bass as bass
import concourse.tile as tile
from concourse import bacc, mybir
from concourse.bass_utils import run_bass_kernel_spmd

dt = mybir.dt
ALU = mybir.AluOpType
AF = mybir.ActivationFunctionType
AX = mybir.AxisListType

B = 128          # graphs
NPER = 512       # nodes per graph
EPER = 512       # hyperedges per graph
DEG = 32         # memberships per node
F = 128          # input feature dim
K = 30           # sortpool k
NCORES = 8
GPC = B // NCORES          # 16 graphs per core
NGROUP = GPC // 4          # 4 groups of 4 graphs
NPAIR = NGROUP // 2        # 2 pairs of groups
C1, C2, KW2 = 16, 32, 5
HDEG = float(DEG + 1)      # node hyperdegree + 1 (structural: 33)

# packed-constant free-dim offsets ([128, CW_TOT] fp32)
_OFF_BDE = 0          # 3 x 128
_OFF_BDN = 384        # 4 x 128
_OFF_IDENT = 896      # 128
_OFF_W0 = 1024        # 32
_OFF_CW1 = 1056       # 4 x 16
_OFF_CW2 = 1120       # 5 x 32
_OFF_OW = 1280        # 2 x 11
_OFF_CB1 = 1302       # 1
_OFF_CB2 = 1303       # 1
_OFF_BEPP = 1304      # 4 x 1
_OFF_BNPP = 1308      # 4 x 1
_OFF_SSUM = 1312      # 4
CW_TOT = 1316

_CACHE = {}


def _pad32(w):
    out = np.zeros((32, 32), np.float32)
    out[: w.shape[0], : w.shape[1]] = w
    return out


def _blockdiag4(w):
    out = np.zeros((128, 128), np.float32)
    for g in range(4):
        out[32 * g : 32 * g + 32, 32 * g : 32 * g + 32] = w
    return out


def _build_program():
    nc = bacc.Bacc("TRN2", target_bir_lowering=False, debug=False,
                   num_devices=NCORES)

    # ---- DRAM I/O ----
    CONST = nc.dram_tensor("constpk", [128, CW_TOT], dt.float32, kind="ExternalInput").ap()
    OUTB = nc.dram_tensor("outb", [4, 8], dt.float32, kind="ExternalInput").ap()
    APK = nc.dram_tensor("apk", [NGROUP, 128, 4, 4, 512], dt.float16, kind="ExternalInput").ap()
    ATPK = nc.dram_tensor("atpk", [NGROUP, 128, 4, 4, 512], dt.float16, kind="ExternalInput").ap()
    FPK = nc.dram_tensor("fpk", [NGROUP, 128, 5, 512], dt.float32, kind="ExternalInput").ap()
    OUT = nc.dram_tensor("out", [GPC, 2], dt.float32, kind="ExternalOutput").ap()
    IDXD = nc.dram_tensor("idxd", [NPAIR, 2, 128, 2], dt.int16, kind="Internal").ap()

    with tile.TileContext(nc) as tc, ExitStack() as ctx:
        cpool = ctx.enter_context(tc.tile_pool(name="consts", bufs=1))
        gpool = ctx.enter_context(tc.tile_pool(name="graph", bufs=1))
        apool = ctx.enter_context(tc.tile_pool(name="amat", bufs=2))
        atpoolA = ctx.enter_context(tc.tile_pool(name="atmatA", bufs=2))
        atpoolB = ctx.enter_context(tc.tile_pool(name="atmatB", bufs=2))
        hpool = ctx.enter_context(tc.tile_pool(name="acts", bufs=2))
        hcatp = ctx.enter_context(tc.tile_pool(name="hcat", bufs=4))
        tpool = ctx.enter_context(tc.tile_pool(name="tmp", bufs=2))
        t3pool = ctx.enter_context(tc.tile_pool(name="tmp3", bufs=3))
        kpool = ctx.enter_context(tc.tile_pool(name="keys", bufs=1))
        pzn = ctx.enter_context(tc.tile_pool(name="pzn", bufs=2, space="PSUM"))
        pagg = ctx.enter_context(tc.tile_pool(name="pagg", bufs=4, space="PSUM"))
        ps2 = ctx.enter_context(tc.tile_pool(name="ps2", bufs=2, space="PSUM"))
        # bank budget (8): pzn x3 + pagg x3 + ps2 x2 = 8

        ct = cpool.tile([128, CW_TOT], dt.float32, tag="constpk")
        nc.sync.dma_start(ct[:], CONST)
        outb = cpool.tile([4, 8], dt.float32, tag="outb")
        nc.sync.dma_start(outb[:], OUTB)

        w0 = ct[:, _OFF_W0 : _OFF_W0 + 32]
        bde = [ct[:, _OFF_BDE + 128 * l : _OFF_BDE + 128 * l + 128] for l in range(3)]
        bdn = [ct[:, _OFF_BDN + 128 * l : _OFF_BDN + 128 * l + 128] for l in range(4)]
        bepp = [ct[:, _OFF_BEPP + l : _OFF_BEPP + l + 1] for l in range(4)]
        bnpp = [ct[:, _OFF_BNPP + l : _OFF_BNPP + l + 1] for l in range(4)]
        ident = ct[:, _OFF_IDENT : _OFF_IDENT + 128]
        cw1 = [ct[:, _OFF_CW1 + 16 * l : _OFF_CW1 + 16 * l + 16] for l in range(4)]
        cb1 = ct[:, _OFF_CB1 : _OFF_CB1 + 1]
        cw2 = [ct[:, _OFF_CW2 + 32 * d : _OFF_CW2 + 32 * d + 32] for d in range(5)]
        cb2 = ct[:, _OFF_CB2 : _OFF_CB2 + 1]
        ow = [ct[:, _OFF_OW + 11 * o : _OFF_OW + 11 * o + 11] for o in range(2)]
        ssum = ct[:, _OFF_SSUM : _OFF_SSUM + 4]

        # pair P uses partitions 32P..32P+8 (engine ops need 32-aligned bases)
        keysB = kpool.tile([64, 512], dt.float32, tag="keysB")
        Yout = kpool.tile([128, 8], dt.float32, tag="yout")

        def direction(gi, l, side, st):
            """One message-passing direction for group-slot gi.

            The linear transform is computed with the activations as the
            stationary operand (lhsT = hT chunk, rhs = block-diag weights), so
            the result lands in PSUM directly in contraction-major layout
            [node/edge partition, (chunk, graph, feature) columns] — no PE
            transpose or PSUM->SBUF staging copy is needed.
            """
            hT_in = st["hT"]
            zN = pzn.tile([128, 512], dt.float32, tag="zn")
            if side == "E" and l == 0:
                for c in range(4):
                    for g in range(4):
                        nc.tensor.matmul(
                            zN[:, 128 * c + 32 * g : 128 * c + 32 * g + 32],
                            st["fpk"][:, g, 128 * c : 128 * c + 128], w0,
                            start=True, stop=True)
            else:
                src = hT_in[:] if side == "E" else st["heT"][:]
                bdw = bde[l - 1] if side == "E" else bdn[l]
                for c in range(4):
                    nc.tensor.matmul(zN[:, 128 * c : 128 * c + 128],
                                     src[:, 128 * c : 128 * c + 128], bdw,
                                     start=True, stop=True)
            zhi = t3pool.tile([128, 512], dt.float16, tag="zhi")
            nc.scalar.copy(zhi[:], zN[:])
            zlo = t3pool.tile([128, 512], dt.float16, tag="zlo")
            nc.vector.tensor_tensor(zlo[:], zN[:], zhi[:], ALU.subtract)

            # aggregation, graph-interleaved issue: the four per-graph
            # accumulation chains target distinct 32-col PE tiles so adjacent
            # matmuls stream concurrently; per-graph PSUM accumulation order
            # (zhi c0..c3 then zlo c0..c3) matches the reference kernel.
            Am = st["A"] if side == "E" else st["At"]
            agg = pagg.tile([128, 512], dt.float32, tag="agg")
            n = 0
            for zp in (zhi, zlo):
                for c in range(4):
                    for g in range(4):
                        nc.tensor.matmul(
                            agg[32 * g : 32 * g + 32, :],
                            zp[:, 128 * c + 32 * g : 128 * c + 32 * g + 32],
                            Am[:, g, c, :], start=(n == 0), stop=(n == 7),
                            tile_position=(0, 32 * g))
                    n += 1
            if side == "E":
                ue = tpool.tile([128, 512], dt.float32, tag="ue")
                nc.vector.scalar_tensor_tensor(ue[:], agg[:], bepp[l], st["recip"][:],
                                               ALU.add, ALU.mult)
                heT = hpool.tile([128, 512], dt.float32, tag="heT")
                nc.scalar.activation(heT[:], ue[:], AF.Tanh)
                st["heT"] = heT
            else:
                hT = hcatp.tile([128, 512], dt.float32, tag=f"hT{l}")
                nc.scalar.activation(hT[:], agg[:], AF.Tanh, bias=bnpp[l],
                                     scale=1.0 / HDEG)
                st["hT"] = hT
                st["hcat"][l] = hT
                if l == 3:
                    r = st["krow"]
                    krows = hT[:].rearrange("(a b) f -> a b f", b=32)[:, 0, :]
                    nc.sync.dma_start(keysB[r : r + 4, :], krows)

        all_states = []
        for P in range(NPAIR):
            states = []
            # issue fpk/A for both groups before the (later-needed) At tiles
            fpks, Amats = [], []
            for j in range(2):
                G = 2 * P + j
                fpk = gpool.tile([128, 5, 512], dt.float32, tag=f"fpk{j}")
                nc.sync.dma_start(fpk[:], FPK[G])
                fpks.append(fpk)
                A = apool.tile([128, 4, 4, 512], dt.float16, tag=f"A{j}")
                nc.sync.dma_start(A[:], APK[G])
                Amats.append(A)
            Atmats = []
            for j in range(2):
                G = 2 * P + j
                atp = atpoolA if j == 0 else atpoolB
                At = atp.tile([128, 4, 4, 512], dt.float16, tag=f"At{j}")
                nc.sync.dma_start(At[:], ATPK[G])
                Atmats.append(At)
            for j in range(2):
                G = 2 * P + j
                fpk, A, At = fpks[j], Amats[j], Atmats[j]
                st = {
                    "G": G,
                    "krow": 32 * P + 4 * j,
                    "fpk": fpk,
                    "A": A[:], "At": At[:],
                    "hcat": [None] * 4,
                    "hT": None, "heT": None,
                }
                recip = hpool.tile([128, 512], dt.float32, tag="recip")
                nc.vector.reciprocal(recip[:], fpk[:, 4, :])
                st["recip"] = recip
                states.append(st)

            for l in range(4):
                for side in ("E", "N"):
                    for j in range(2):
                        direction(j, l, side, states[j])

            # ---- per-pair top-k (partitions 32P..32P+8 of the key tiles) ----
            r0 = 32 * P
            kw = keysB   # destructive top-k: keys are not needed afterwards
            idxu = kpool.tile([64, 32], dt.uint32, tag="idxu")
            for r in range(4):
                m8 = kpool.tile([64, 8], dt.float32, tag="m8")
                nc.vector.max(m8[r0 : r0 + 8, :], kw[r0 : r0 + 8, :])
                nc.vector.max_index(idxu[r0 : r0 + 8, 8 * r : 8 * r + 8],
                                    m8[r0 : r0 + 8, :], kw[r0 : r0 + 8, :])
                nc.vector.match_replace(kw[r0 : r0 + 8, :], m8[r0 : r0 + 8, :],
                                        kw[r0 : r0 + 8, :], -1e30)
            idx16 = kpool.tile([64, 32], dt.int16, tag="idx16")
            nc.vector.tensor_copy(idx16[r0 : r0 + 8, :], idxu[r0 : r0 + 8, :])
            # pre-wrap into ap_gather layout: row m becomes 2x-replicated
            # (idx[0], idx[16], idx[1], idx[17], ...) so a plain partition-
            # scatter DMA lands idx i at partition i%16, col i//16
            idx16i = kpool.tile([64, 64], dt.int16, tag="idx16i")
            wsrc = idx16[r0 : r0 + 8, :].rearrange("m (t lo) -> m lo t", lo=16) \
                .unsqueeze(1).broadcast_to([8, 2, 16, 2])
            wdst = idx16i[r0 : r0 + 8, :].rearrange("m (s lo t) -> m s lo t", s=2, t=2)
            nc.vector.tensor_copy(wdst, wsrc)
            for j in range(2):
                nc.sync.dma_start(
                    IDXD[P, j].rearrange("(m p) t -> m (p t)", m=4),
                    idx16i[r0 + 4 * j : r0 + 4 * j + 4, :])

            # ---- pooled gather + conv tower per group of this pair ----
            for j in range(2):
                G = 2 * P + j
                tiles = states[j]["hcat"]
                idxw = tpool.tile([128, 2], dt.int16, tag="idxw")
                nc.sync.dma_start(idxw[:], IDXD[P, j])

                pgs = []
                for l in range(4):
                    pg = tpool.tile([128, 32], dt.float32, tag=f"pg{l}")
                    nc.gpsimd.ap_gather(pg[:], tiles[l][:].unsqueeze(2), idxw[:],
                                        channels=128, num_elems=512, d=1, num_idxs=32)
                    pgs.append(pg)

                y1 = ps2.tile([128, 30], dt.float32, tag="small")
                for g in range(4):
                    for l in range(4):
                        nc.tensor.matmul(y1[32 * g : 32 * g + 16, :],
                                         cw1[l][32 * g : 32 * g + 32, :],
                                         pgs[l][32 * g : 32 * g + 32, 0:30],
                                         start=(l == 0), stop=(l == 3),
                                         tile_position=(32 * g, 32 * g))
                y1r = tpool.tile([128, 30], dt.float32, tag="y1r")
                nc.scalar.activation(y1r[:], y1[:], AF.Relu, bias=cb1)
                y1p = tpool.tile([128, 15], dt.float32, tag="y1p")
                nc.vector.tensor_tensor(
                    y1p[:], y1r[:].rearrange("p (t two) -> p t two", two=2)[:, :, 0],
                    y1r[:].rearrange("p (t two) -> p t two", two=2)[:, :, 1], ALU.max)

                y2 = ps2.tile([128, 11], dt.float32, tag="small")
                for g in range(4):
                    for d in range(5):
                        nc.tensor.matmul(y2[32 * g : 32 * g + 32, :],
                                         cw2[d][32 * g : 32 * g + 32, :],
                                         y1p[32 * g : 32 * g + 32, d : d + 11],
                                         start=(d == 0), stop=(d == 4),
                                         tile_position=(32 * g, 32 * g))
                y2r = tpool.tile([128, 11], dt.float32, tag="y2r")
                nc.scalar.activation(y2r[:], y2[:], AF.Relu, bias=cb2)
                for o in range(2):
                    t_o = tpool.tile([128, 11], dt.float32, tag="t_o")
                    nc.vector.tensor_tensor(t_o[:], y2r[:], ow[o], ALU.mult)
                    nc.vector.tensor_reduce(Yout[:, 2 * G + o : 2 * G + o + 1],
                                            t_o[:], AX.X, ALU.add)

        # ---------- final dense + relu + output ----------
        pout = ps2.tile([4, 8], dt.float32, tag="small")
        nc.tensor.matmul(pout[:], ssum, Yout[:], start=True, stop=True)
        ob = kpool.tile([4, 8], dt.float32, tag="ob")
        nc.vector.tensor_tensor(ob[:], pout[:], outb[:], ALU.add)
        orl = kpool.tile([4, 8], dt.float32, tag="orl")
        nc.scalar.activation(orl[:], ob[:], AF.Relu)
        nc.sync.dma_start(OUT.rearrange("(G g) o -> g G o", g=4), orl[:])

    nc.compile()
    return nc


def _make_consts(inputs):
    ws = [inputs[f"w{i}"].astype(np.float32) for i in range(8)]
    bs = [inputs[f"b{i}"].astype(np.float32) for i in range(8)]
    wE = [ws[0], _pad32(ws[2]), _pad32(ws[4]), _pad32(ws[6])]
    wN = [_pad32(ws[1]), _pad32(ws[3]), _pad32(ws[5]), _pad32(ws[7])]
    bE = [bs[0], bs[2], bs[4], np.pad(bs[6], (0, 31))]
    bN = [bs[1], bs[3], bs[5], np.pad(bs[7], (0, 31))]

    cpk = np.zeros((128, CW_TOT), np.float32)
    for l in range(1, 4):
        cpk[:, _OFF_BDE + 128 * (l - 1) : _OFF_BDE + 128 * l] = _blockdiag4(wE[l])
    for l in range(4):
        cpk[:, _OFF_BDN + 128 * l : _OFF_BDN + 128 * (l + 1)] = _blockdiag4(wN[l])
    cpk[:, _OFF_IDENT : _OFF_IDENT + 128] = np.eye(128, dtype=np.float32)
    cpk[:, _OFF_W0 : _OFF_W0 + 32] = ws[0]
    for l in range(4):
        cpk[:, _OFF_BEPP + l] = np.tile(bE[l], 4)
        cpk[:, _OFF_BNPP + l] = np.tile(bN[l], 4) / HDEG

    c1w = inputs["conv1_w"].astype(np.float32).reshape(C1, 97)    # [16, 97]
    for l in range(4):
        blk = np.zeros((32, 16), np.float32)
        if l < 3:
            blk = c1w[:, 32 * l : 32 * l + 32].T
        else:
            blk[0, :] = c1w[:, 96]
        for g in range(4):
            cpk[32 * g : 32 * g + 32, _OFF_CW1 + 16 * l : _OFF_CW1 + 16 * (l + 1)] = blk
    for g in range(4):
        cpk[32 * g : 32 * g + 16, _OFF_CB1] = inputs["conv1_b"]
    c2w = inputs["conv2_w"].astype(np.float32)                    # [32, 16, 5]
    for d in range(5):
        for g in range(4):
            cpk[32 * g : 32 * g + 16, _OFF_CW2 + 32 * d : _OFF_CW2 + 32 * (d + 1)] = c2w[:, :, d].T
    for g in range(4):
        cpk[32 * g : 32 * g + 32, _OFF_CB2] = inputs["conv2_b"]
    oww = inputs["out_w"].astype(np.float32)                      # [352, 2]
    for o in range(2):
        for g in range(4):
            cpk[32 * g : 32 * g + 32, _OFF_OW + 11 * o : _OFF_OW + 11 * (o + 1)] = \
                oww[:, o].reshape(C2, 11)
    for j in range(4):
        cpk[32 * j : 32 * j + 32, _OFF_SSUM + j] = 1.0

    outb = np.tile(inputs["out_b"].astype(np.float32), (4, 4))    # [4, 8]
    return {"constpk": cpk, "outb": outb}


def _build_incidence(inputs):
    """Host-side A / At / hyperedge-size construction (exact small ints)."""
    einc_g = np.asarray(inputs["inc_edge"]).reshape(B, NPER, DEG).astype(np.int64)
    base = (np.arange(B, dtype=np.int64) * EPER)[:, None, None]
    loc = einc_g - base                                           # [B, 512, 32] local
    A_all = np.empty((B, 128, 4, EPER), np.float16)
    At_all = np.empty((B, 128, 4, NPER), np.float16)
    hs_all = np.empty((B, EPER), np.float32)
    rowbase = (np.arange(NPER, dtype=np.int64) * EPER)[:, None]
    for g in range(B):
        flat = (rowbase + loc[g]).ravel()
        cnt = np.bincount(flat, minlength=NPER * EPER).reshape(NPER, EPER)
        c16 = cnt.astype(np.float16)                              # [n, e]
        A_all[g] = c16.reshape(4, 128, EPER).transpose(1, 0, 2)
        At_all[g] = np.ascontiguousarray(c16.T).reshape(4, 128, NPER).transpose(1, 0, 2)
        hs_all[g] = cnt.sum(axis=0).astype(np.float32) + 1.0
    return A_all, At_all, hs_all


def make_in_maps(inputs):
    consts = _make_consts(inputs)
    nf = np.asarray(inputs["node_feat"]).astype(np.float32).reshape(B, NPER, F)
    nfT = nf.transpose(0, 2, 1)                                   # [B, 128f, 512n]
    A_all, At_all, hs_all = _build_incidence(inputs)
    in_maps = []
    for c in range(NCORES):
        m = dict(consts)
        sl = slice(c * GPC, (c + 1) * GPC)
        # apk/atpk: [NGROUP, 128, 4g, 4c, 512]
        m["apk"] = np.ascontiguousarray(
            A_all[sl].reshape(NGROUP, 4, 128, 4, EPER).transpose(0, 2, 1, 3, 4))
        m["atpk"] = np.ascontiguousarray(
            At_all[sl].reshape(NGROUP, 4, 128, 4, NPER).transpose(0, 2, 1, 3, 4))
        # fpk: [NGROUP, 128, 5, 512] = 4 transposed-feature graphs + hsize row
        fpk = np.empty((NGROUP, 128, 5, 512), np.float32)
        nfT_c = nfT[sl].reshape(NGROUP, 4, 128, NPER)
        for G in range(NGROUP):
            for g in range(4):
                fpk[G, :, g, :] = nfT_c[G, g]
        hs_core = hs_all[sl].reshape(NGROUP, 4, EPER)
        fpk[:, :, 4, :] = np.repeat(hs_core[:, :, None, :], 32, axis=2).reshape(
            NGROUP, 128, EPER)
        m["fpk"] = np.ascontiguousarray(fpk)
        in_maps.append(m)
    return in_maps


def get_program():
    if "nc" not in _CACHE:
        _CACHE["nc"] = _build_program()
    return _CACHE["nc"]


def kernel(**inputs):
    nc = get_program()
    in_maps = make_in_maps(inputs)
    res = run_bass_kernel_spmd(nc, in_maps, core_ids=list(range(NCORES)))
    out = np.concatenate([res.results[c]["out"] for c in range(NCORES)], axis=0)
    return out.astype(np.float32)


# revision 43
# speedup vs baseline: 10252.1952x; 1.0412x over previous
"""DGCNN hypergraph kernel for Trainium2 (Bass/Tile), 8-core SPMD.

Strategy (per the data-parallel sharding hint): 128 disjoint hypergraphs are
sharded 16-per-core across 8 NeuronCores. All message passing is graph-local.

The incidence matrices A (node->edge counts), their transposes At, and the
hyperedge sizes are pure functions of the integer incidence input, so they are
built on the host (exact small-integer fp16) and DMAed in; the device runs only
the floating-point pipeline.

Per-core pipeline (16 graphs, processed as 2 pairs-of-groups; the two groups of
a pair are interleaved sub-stage by sub-stage so their dependency chains
overlap on all engines with only bufs=2 PSUM pools):
  - 4 conv layers x 2 directions per group: linear (fp32 matmul, block-diag
    weights for 4-graph batching), PE transpose to node-major, fp16 hi/lo pair
    split, and aggregation as col-tiled fp16 matmuls against A / At accumulated
    in PSUM (2-pass hi/lo gives ~fp32 accuracy), then bias/degree-scale + tanh.
    Aggregation matmuls are issued graph-interleaved so the four 32-column PE
    tile chains stream concurrently.
  - Sort-pooling per pair: top-30 per graph via max8/max_index/match_replace
    rounds (tie behavior matches jax stable top_k), gather via ap_gather.
  - Conv tower + dense layer via small fp32 matmuls, relu, output assembly.
"""

import numpy as np
from contextlib import ExitStack

import concourse.bass as bass
import concourse.tile as tile
from concourse import bacc, mybir
from concourse.bass_utils import run_bass_kernel_spmd

dt = mybir.dt
ALU = mybir.AluOpType
AF = mybir.ActivationFunctionType
AX = mybir.AxisListType

B = 128          # graphs
NPER = 512       # nodes per graph
EPER = 512       # hyperedges per graph
DEG = 32         # memberships per node
F = 128          # input feature dim
K = 30           # sortpool k
NCORES = 8
GPC = B // NCORES          # 16 graphs per core
NGROUP = GPC // 4          # 4 groups of 4 graphs
NPAIR = NGROUP // 2        # 2 pairs of groups
C1, C2, KW2 = 16, 32, 5
HDEG = float(DEG + 1)      # node hyperdegree + 1 (structural: 33)

# packed-constant free-dim offsets ([128, CW_TOT] fp32)
_OFF_BDE = 0          # 3 x 128
_OFF_BDN = 384        # 4 x 128
_OFF_IDENT = 896      # 128
_OFF_W0 = 1024        # 32
_OFF_CW1 = 1056       # 4 x 16
_OFF_CW2 = 1120       # 5 x 32
_OFF_OW = 1280        # 2 x 11
_OFF_CB1 = 1302       # 1
_OFF_CB2 = 1303       # 1
_OFF_BEPP = 1304      # 4 x 1
_OFF_BNPP = 1308      # 4 x 1
_OFF_SSUM = 1312      # 4
CW_TOT = 1316

_CACHE = {}


def _pad32(w):
    out = np.zeros((32, 32), np.float32)
    out[: w.shape[0], : w.shape[1]] = w
    return out


def _blockdiag4(w):
    out = np.zeros((128, 128), np.float32)
    for g in range(4):
        out[32 * g : 32 * g + 32, 32 * g : 32 * g + 32] = w
    return out


def _build_program():
    nc = bacc.Bacc("TRN2", target_bir_lowering=False, debug=False,
                   num_devices=NCORES)

    # ---- DRAM I/O ----
    CONST = nc.dram_tensor("constpk", [128, CW_TOT], dt.float32, kind="ExternalInput").ap()
    OUTB = nc.dram_tensor("outb", [4, 8], dt.float32, kind="ExternalInput").ap()
    APK = nc.dram_tensor("apk", [NGROUP, 128, 4, 4, 512], dt.float16, kind="ExternalInput").ap()
    ATPK = nc.dram_tensor("atpk", [NGROUP, 128, 4, 4, 512], dt.float16, kind="ExternalInput").ap()
    FPK = nc.dram_tensor("fpk", [NGROUP, 128, 5, 512], dt.float32, kind="ExternalInput").ap()
    OUT = nc.dram_tensor("out", [GPC, 2], dt.float32, kind="ExternalOutput").ap()
    IDXD = nc.dram_tensor("idxd", [NPAIR, 2, 128, 2], dt.int16, kind="Internal").ap()

    with tile.TileContext(nc) as tc, ExitStack() as ctx:
        cpool = ctx.enter_context(tc.tile_pool(name="consts", bufs=1))
        gpool = ctx.enter_context(tc.tile_pool(name="graph", bufs=1))
        apool = ctx.enter_context(tc.tile_pool(name="amat", bufs=2))
        atpoolA = ctx.enter_context(tc.tile_pool(name="atmatA", bufs=2))
        atpoolB = ctx.enter_context(tc.tile_pool(name="atmatB", bufs=2))
        hpool = ctx.enter_context(tc.tile_pool(name="acts", bufs=2))
        hcatp = ctx.enter_context(tc.tile_pool(name="hcat", bufs=4))
        tpool = ctx.enter_context(tc.tile_pool(name="tmp", bufs=2))
        t3pool = ctx.enter_context(tc.tile_pool(name="tmp3", bufs=3))
        kpool = ctx.enter_context(tc.tile_pool(name="keys", bufs=1))
        pzn = ctx.enter_context(tc.tile_pool(name="pzn", bufs=2, space="PSUM"))
        pagg = ctx.enter_context(tc.tile_pool(name="pagg", bufs=4, space="PSUM"))
        ps2 = ctx.enter_context(tc.tile_pool(name="ps2", bufs=2, space="PSUM"))
        # bank budget (8): pzn x3 + pagg x3 + ps2 x2 = 8

        ct = cpool.tile([128, CW_TOT], dt.float32, tag="constpk")
        nc.sync.dma_start(ct[:], CONST)
        outb = cpool.tile([4, 8], dt.float32, tag="outb")
        nc.sync.dma_start(outb[:], OUTB)

        w0 = ct[:, _OFF_W0 : _OFF_W0 + 32]
        bde = [ct[:, _OFF_BDE + 128 * l : _OFF_BDE + 128 * l + 128] for l in range(3)]
        bdn = [ct[:, _OFF_BDN + 128 * l : _OFF_BDN + 128 * l + 128] for l in range(4)]
        bepp = [ct[:, _OFF_BEPP + l : _OFF_BEPP + l + 1] for l in range(4)]
        bnpp = [ct[:, _OFF_BNPP + l : _OFF_BNPP + l + 1] for l in range(4)]
        ident = ct[:, _OFF_IDENT : _OFF_IDENT + 128]
        cw1 = [ct[:, _OFF_CW1 + 16 * l : _OFF_CW1 + 16 * l + 16] for l in range(4)]
        cb1 = ct[:, _OFF_CB1 : _OFF_CB1 + 1]
        cw2 = [ct[:, _OFF_CW2 + 32 * d : _OFF_CW2 + 32 * d + 32] for d in range(5)]
        cb2 = ct[:, _OFF_CB2 : _OFF_CB2 + 1]
        ow = [ct[:, _OFF_OW + 11 * o : _OFF_OW + 11 * o + 11] for o in range(2)]
        ssum = ct[:, _OFF_SSUM : _OFF_SSUM + 4]

        # pair P uses partitions 32P..32P+8 (engine ops need 32-aligned bases)
        keysB = kpool.tile([64, 512], dt.float32, tag="keysB")
        Yout = kpool.tile([128, 8], dt.float32, tag="yout")

        def direction(gi, l, side, st):
            """One message-passing direction for group-slot gi.

            The linear transform is computed with the activations as the
            stationary operand (lhsT = hT chunk, rhs = block-diag weights), so
            the result lands in PSUM directly in contraction-major layout
            [node/edge partition, (chunk, graph, feature) columns] — no PE
            transpose or PSUM->SBUF staging copy is needed.
            """
            hT_in = st["hT"]
            zN = pzn.tile([128, 512], dt.float32, tag="zn")
            if side == "E" and l == 0:
                for c in range(4):
                    for g in range(4):
                        nc.tensor.matmul(
                            zN[:, 128 * c + 32 * g : 128 * c + 32 * g + 32],
                            st["fpk"][:, g, 128 * c : 128 * c + 128], w0,
                            start=True, stop=True)
            else:
                src = hT_in[:] if side == "E" else st["heT"][:]
                bdw = bde[l - 1] if side == "E" else bdn[l]
                for c in range(4):
                    nc.tensor.matmul(zN[:, 128 * c : 128 * c + 128],
                                     src[:, 128 * c : 128 * c + 128], bdw,
                                     start=True, stop=True)
            zhi = t3pool.tile([128, 512], dt.float16, tag="zhi")
            nc.scalar.copy(zhi[:], zN[:])
            zlo = t3pool.tile([128, 512], dt.float16, tag="zlo")
            nc.vector.tensor_tensor(zlo[:], zN[:], zhi[:], ALU.subtract)

            # aggregation, graph-interleaved issue: the four per-graph
            # accumulation chains target distinct 32-col PE tiles so adjacent
            # matmuls stream concurrently; per-graph PSUM accumulation order
            # (zhi c0..c3 then zlo c0..c3) matches the reference kernel.
            Am = st["A"] if side == "E" else st["At"]
            agg = pagg.tile([128, 512], dt.float32, tag="agg")
            n = 0
            for zp in (zhi, zlo):
                for c in range(4):
                    for g in range(4):
                        nc.tensor.matmul(
                            agg[32 * g : 32 * g + 32, :],
                            zp[:, 128 * c + 32 * g : 128 * c + 32 * g + 32],
                            Am[:, g, c, :], start=(n == 0), stop=(n == 7),
                            tile_position=(0, 32 * g))
                    n += 1
            if side == "E":
                ue = tpool.tile([128, 512], dt.float32, tag="ue")
                nc.vector.scalar_tensor_tensor(ue[:], agg[:], bepp[l], st["recip"][:],
                                               ALU.add, ALU.mult)
                heT = hpool.tile([128, 512], dt.float32, tag="heT")
                nc.scalar.activation(heT[:], ue[:], AF.Tanh)
                st["heT"] = heT
            else:
                hT = hcatp.tile([128, 512], dt.float32, tag=f"hT{l}")
                nc.scalar.activation(hT[:], agg[:], AF.Tanh, bias=bnpp[l],
                                     scale=1.0 / HDEG)
                st["hT"] = hT
                st["hcat"][l] = hT
                if l == 3:
                    r = st["krow"]
                    krows = hT[:].rearrange("(a b) f -> a b f", b=32)[:, 0, :]
                    nc.sync.dma_start(keysB[r : r + 4, :], krows)

        all_states = []
        for P in range(NPAIR):
            states = []
            # issue fpk/A for both groups before the (later-needed) At tiles
            fpks, Amats = [], []
            for j in range(2):
                G = 2 * P + j
                fpk = gpool.tile([128, 5, 512], dt.float32, tag=f"fpk{j}")
                nc.sync.dma_start(fpk[:], FPK[G])
                fpks.append(fpk)
                A = apool.tile([128, 4, 4, 512], dt.float16, tag=f"A{j}")
                nc.sync.dma_start(A[:], APK[G])
                Amats.append(A)
            for j in range(2):
                G = 2 * P + j
                fpk, A = fpks[j], Amats[j]
                st = {
                    "G": G,
                    "krow": 32 * P + 4 * j,
                    "fpk": fpk,
                    "A": A[:], "At": None,
                    "hcat": [None] * 4,
                    "hT": None, "heT": None,
                }
                recip = hpool.tile([128, 512], dt.float32, tag="recip")
                nc.vector.tensor_copy(recip[:], fpk[:, 4, :])
                st["recip"] = recip
                states.append(st)

            # E-side of layer 0 first: its DMAs (fpk/A) are the critical path,
            # so the At transfers are issued only afterwards
            for j in range(2):
                direction(j, 0, "E", states[j])
            for j in range(2):
                G = 2 * P + j
                atp = atpoolA if j == 0 else atpoolB
                At = atp.tile([128, 4, 4, 512], dt.float16, tag=f"At{j}")
                nc.sync.dma_start(At[:], ATPK[G])
                states[j]["At"] = At[:]
            for j in range(2):
                direction(j, 0, "N", states[j])
            for l in range(1, 4):
                for side in ("E", "N"):
                    for j in range(2):
                        direction(j, l, side, states[j])

            # ---- per-pair top-k (partitions 32P..32P+8 of the key tiles) ----
            r0 = 32 * P
            kw = keysB   # destructive top-k: keys are not needed afterwards
            idxu = kpool.tile([64, 32], dt.uint32, tag="idxu")
            for r in range(4):
                m8 = kpool.tile([64, 8], dt.float32, tag="m8")
                nc.vector.max(m8[r0 : r0 + 8, :], kw[r0 : r0 + 8, :])
                nc.vector.max_index(idxu[r0 : r0 + 8, 8 * r : 8 * r + 8],
                                    m8[r0 : r0 + 8, :], kw[r0 : r0 + 8, :])
                nc.vector.match_replace(kw[r0 : r0 + 8, :], m8[r0 : r0 + 8, :],
                                        kw[r0 : r0 + 8, :], -1e30)
            idx16 = kpool.tile([64, 32], dt.int16, tag="idx16")
            nc.vector.tensor_copy(idx16[r0 : r0 + 8, :], idxu[r0 : r0 + 8, :])
            # pre-wrap into ap_gather layout: row m becomes 2x-replicated
            # (idx[0], idx[16], idx[1], idx[17], ...) so a plain partition-
            # scatter DMA lands idx i at partition i%16, col i//16
            idx16i = kpool.tile([64, 64], dt.int16, tag="idx16i")
            wsrc = idx16[r0 : r0 + 8, :].rearrange("m (t lo) -> m lo t", lo=16) \
                .unsqueeze(1).broadcast_to([8, 2, 16, 2])
            wdst = idx16i[r0 : r0 + 8, :].rearrange("m (s lo t) -> m s lo t", s=2, t=2)
            nc.vector.tensor_copy(wdst, wsrc)
            for j in range(2):
                nc.sync.dma_start(
                    IDXD[P, j].rearrange("(m p) t -> m (p t)", m=4),
                    idx16i[r0 + 4 * j : r0 + 4 * j + 4, :])

            # ---- pooled gather + conv tower per group of this pair ----
            for j in range(2):
                G = 2 * P + j
                tiles = states[j]["hcat"]
                idxw = tpool.tile([128, 2], dt.int16, tag="idxw")
                nc.sync.dma_start(idxw[:], IDXD[P, j])

                pgs = []
                for l in range(4):
                    pg = tpool.tile([128, 32], dt.float32, tag=f"pg{l}")
                    nc.gpsimd.ap_gather(pg[:], tiles[l][:].unsqueeze(2), idxw[:],
                                        channels=128, num_elems=512, d=1, num_idxs=32)
                    pgs.append(pg)

                y1 = ps2.tile([128, 30], dt.float32, tag="small")
                for g in range(4):
                    for l in range(4):
                        nc.tensor.matmul(y1[32 * g : 32 * g + 16, :],
                                         cw1[l][32 * g : 32 * g + 32, :],
                                         pgs[l][32 * g : 32 * g + 32, 0:30],
                                         start=(l == 0), stop=(l == 3),
                                         tile_position=(32 * g, 32 * g))
                y1r = tpool.tile([128, 30], dt.float32, tag="y1r")
                nc.scalar.activation(y1r[:], y1[:], AF.Relu, bias=cb1)
                y1p = tpool.tile([128, 15], dt.float32, tag="y1p")
                nc.vector.tensor_tensor(
                    y1p[:], y1r[:].rearrange("p (t two) -> p t two", two=2)[:, :, 0],
                    y1r[:].rearrange("p (t two) -> p t two", two=2)[:, :, 1], ALU.max)

                y2 = ps2.tile([128, 11], dt.float32, tag="small")
                for g in range(4):
                    for d in range(5):
                        nc.tensor.matmul(y2[32 * g : 32 * g + 32, :],
                                         cw2[d][32 * g : 32 * g + 32, :],
                                         y1p[32 * g : 32 * g + 32, d : d + 11],
                                         start=(d == 0), stop=(d == 4),
                                         tile_position=(32 * g, 32 * g))
                y2r = tpool.tile([128, 11], dt.float32, tag="y2r")
                nc.scalar.activation(y2r[:], y2[:], AF.Relu, bias=cb2)
                for o in range(2):
                    t_o = tpool.tile([128, 11], dt.float32, tag="t_o")
                    nc.vector.tensor_tensor(t_o[:], y2r[:], ow[o], ALU.mult)
                    nc.vector.tensor_reduce(Yout[:, 2 * G + o : 2 * G + o + 1],
                                            t_o[:], AX.X, ALU.add)

        # ---------- final dense + relu + output ----------
        pout = ps2.tile([4, 8], dt.float32, tag="small")
        nc.tensor.matmul(pout[:], ssum, Yout[:], start=True, stop=True)
        ob = kpool.tile([4, 8], dt.float32, tag="ob")
        nc.vector.tensor_tensor(ob[:], pout[:], outb[:], ALU.add)
        orl = kpool.tile([4, 8], dt.float32, tag="orl")
        nc.scalar.activation(orl[:], ob[:], AF.Relu)
        nc.sync.dma_start(OUT.rearrange("(G g) o -> g G o", g=4), orl[:])

    nc.compile()
    return nc


def _make_consts(inputs):
    ws = [inputs[f"w{i}"].astype(np.float32) for i in range(8)]
    bs = [inputs[f"b{i}"].astype(np.float32) for i in range(8)]
    wE = [ws[0], _pad32(ws[2]), _pad32(ws[4]), _pad32(ws[6])]
    wN = [_pad32(ws[1]), _pad32(ws[3]), _pad32(ws[5]), _pad32(ws[7])]
    bE = [bs[0], bs[2], bs[4], np.pad(bs[6], (0, 31))]
    bN = [bs[1], bs[3], bs[5], np.pad(bs[7], (0, 31))]

    cpk = np.zeros((128, CW_TOT), np.float32)
    for l in range(1, 4):
        cpk[:, _OFF_BDE + 128 * (l - 1) : _OFF_BDE + 128 * l] = _blockdiag4(wE[l])
    for l in range(4):
        cpk[:, _OFF_BDN + 128 * l : _OFF_BDN + 128 * (l + 1)] = _blockdiag4(wN[l])
    cpk[:, _OFF_IDENT : _OFF_IDENT + 128] = np.eye(128, dtype=np.float32)
    cpk[:, _OFF_W0 : _OFF_W0 + 32] = ws[0]
    for l in range(4):
        cpk[:, _OFF_BEPP + l] = np.tile(bE[l], 4)
        cpk[:, _OFF_BNPP + l] = np.tile(bN[l], 4) / HDEG

    c1w = inputs["conv1_w"].astype(np.float32).reshape(C1, 97)    # [16, 97]
    for l in range(4):
        blk = np.zeros((32, 16), np.float32)
        if l < 3:
            blk = c1w[:, 32 * l : 32 * l + 32].T
        else:
            blk[0, :] = c1w[:, 96]
        for g in range(4):
            cpk[32 * g : 32 * g + 32, _OFF_CW1 + 16 * l : _OFF_CW1 + 16 * (l + 1)] = blk
    for g in range(4):
        cpk[32 * g : 32 * g + 16, _OFF_CB1] = inputs["conv1_b"]
    c2w = inputs["conv2_w"].astype(np.float32)                    # [32, 16, 5]
    for d in range(5):
        for g in range(4):
            cpk[32 * g : 32 * g + 16, _OFF_CW2 + 32 * d : _OFF_CW2 + 32 * (d + 1)] = c2w[:, :, d].T
    for g in range(4):
        cpk[32 * g : 32 * g + 32, _OFF_CB2] = inputs["conv2_b"]
    oww = inputs["out_w"].astype(np.float32)                      # [352, 2]
    for o in range(2):
        for g in range(4):
            cpk[32 * g : 32 * g + 32, _OFF_OW + 11 * o : _OFF_OW + 11 * (o + 1)] = \
                oww[:, o].reshape(C2, 11)
    for j in range(4):
        cpk[32 * j : 32 * j + 32, _OFF_SSUM + j] = 1.0

    outb = np.tile(inputs["out_b"].astype(np.float32), (4, 4))    # [4, 8]
    return {"constpk": cpk, "outb": outb}


def _build_incidence(inputs):
    """Host-side A / At / hyperedge-size construction (exact small ints)."""
    einc_g = np.asarray(inputs["inc_edge"]).reshape(B, NPER, DEG).astype(np.int64)
    base = (np.arange(B, dtype=np.int64) * EPER)[:, None, None]
    loc = einc_g - base                                           # [B, 512, 32] local
    A_all = np.empty((B, 128, 4, EPER), np.float16)
    At_all = np.empty((B, 128, 4, NPER), np.float16)
    hs_all = np.empty((B, EPER), np.float32)
    rowbase = (np.arange(NPER, dtype=np.int64) * EPER)[:, None]
    for g in range(B):
        flat = (rowbase + loc[g]).ravel()
        cnt = np.bincount(flat, minlength=NPER * EPER).reshape(NPER, EPER)
        c16 = cnt.astype(np.float16)                              # [n, e]
        A_all[g] = c16.reshape(4, 128, EPER).transpose(1, 0, 2)
        At_all[g] = np.ascontiguousarray(c16.T).reshape(4, 128, NPER).transpose(1, 0, 2)
        hs_all[g] = cnt.sum(axis=0).astype(np.float32) + 1.0
    return A_all, At_all, hs_all


def make_in_maps(inputs):
    consts = _make_consts(inputs)
    nf = np.asarray(inputs["node_feat"]).astype(np.float32).reshape(B, NPER, F)
    nfT = nf.transpose(0, 2, 1)                                   # [B, 128f, 512n]
    A_all, At_all, hs_all = _build_incidence(inputs)
    in_maps = []
    for c in range(NCORES):
        m = dict(consts)
        sl = slice(c * GPC, (c + 1) * GPC)
        # apk/atpk: [NGROUP, 128, 4g, 4c, 512]
        m["apk"] = np.ascontiguousarray(
            A_all[sl].reshape(NGROUP, 4, 128, 4, EPER).transpose(0, 2, 1, 3, 4))
        m["atpk"] = np.ascontiguousarray(
            At_all[sl].reshape(NGROUP, 4, 128, 4, NPER).transpose(0, 2, 1, 3, 4))
        # fpk: [NGROUP, 128, 5, 512] = 4 transposed-feature graphs + hsize row
        fpk = np.empty((NGROUP, 128, 5, 512), np.float32)
        nfT_c = nfT[sl].reshape(NGROUP, 4, 128, NPER)
        for G in range(NGROUP):
            for g in range(4):
                fpk[G, :, g, :] = nfT_c[G, g]
        hs_core = np.divide(np.float32(1.0), hs_all[sl],
                            dtype=np.float32).reshape(NGROUP, 4, EPER)
        fpk[:, :, 4, :] = np.repeat(hs_core[:, :, None, :], 32, axis=2).reshape(
            NGROUP, 128, EPER)
        m["fpk"] = np.ascontiguousarray(fpk)
        in_maps.append(m)
    return in_maps


def get_program():
    if "nc" not in _CACHE:
        _CACHE["nc"] = _build_program()
    return _CACHE["nc"]


def kernel(**inputs):
    nc = get_program()
    in_maps = make_in_maps(inputs)
    res = run_bass_kernel_spmd(nc, in_maps, core_ids=list(range(NCORES)))
    out = np.concatenate([res.results[c]["out"] for c in range(NCORES)], axis=0)
    return out.astype(np.float32)


# revision 44
# speedup vs baseline: 10302.7775x; 1.0049x over previous
"""DGCNN hypergraph kernel for Trainium2 (Bass/Tile), 8-core SPMD.

Strategy (per the data-parallel sharding hint): 128 disjoint hypergraphs are
sharded 16-per-core across 8 NeuronCores. All message passing is graph-local.

The incidence matrices A (node->edge counts), their transposes At, and the
hyperedge sizes are pure functions of the integer incidence input, so they are
built on the host (exact small-integer fp16) and DMAed in; the device runs only
the floating-point pipeline.

Per-core pipeline (16 graphs, processed as 2 pairs-of-groups; the two groups of
a pair are interleaved sub-stage by sub-stage so their dependency chains
overlap on all engines with only bufs=2 PSUM pools):
  - 4 conv layers x 2 directions per group: linear (fp32 matmul, block-diag
    weights for 4-graph batching), PE transpose to node-major, fp16 hi/lo pair
    split, and aggregation as col-tiled fp16 matmuls against A / At accumulated
    in PSUM (2-pass hi/lo gives ~fp32 accuracy), then bias/degree-scale + tanh.
    Aggregation matmuls are issued graph-interleaved so the four 32-column PE
    tile chains stream concurrently.
  - Sort-pooling per pair: top-30 per graph via max8/max_index/match_replace
    rounds (tie behavior matches jax stable top_k), gather via ap_gather.
  - Conv tower + dense layer via small fp32 matmuls, relu, output assembly.
"""

import numpy as np
from contextlib import ExitStack

import concourse.bass as bass
import concourse.tile as tile
from concourse import bacc, mybir
from concourse.bass_utils import run_bass_kernel_spmd

dt = mybir.dt
ALU = mybir.AluOpType
AF = mybir.ActivationFunctionType
AX = mybir.AxisListType

B = 128          # graphs
NPER = 512       # nodes per graph
EPER = 512       # hyperedges per graph
DEG = 32         # memberships per node
F = 128          # input feature dim
K = 30           # sortpool k
NCORES = 8
GPC = B // NCORES          # 16 graphs per core
NGROUP = GPC // 4          # 4 groups of 4 graphs
NPAIR = NGROUP // 2        # 2 pairs of groups
C1, C2, KW2 = 16, 32, 5
HDEG = float(DEG + 1)      # node hyperdegree + 1 (structural: 33)

# packed-constant free-dim offsets ([128, CW_TOT] fp32)
_OFF_BDE = 0          # 3 x 128
_OFF_BDN = 384        # 4 x 128
_OFF_IDENT = 896      # 128
_OFF_W0 = 1024        # 32
_OFF_CW1 = 1056       # 4 x 16
_OFF_CW2 = 1120       # 5 x 32
_OFF_OW = 1280        # 2 x 11
_OFF_CB1 = 1302       # 1
_OFF_CB2 = 1303       # 1
_OFF_BEPP = 1304      # 4 x 1
_OFF_BNPP = 1308      # 4 x 1
_OFF_SSUM = 1312      # 4
CW_TOT = 1316

_CACHE = {}


def _pad32(w):
    out = np.zeros((32, 32), np.float32)
    out[: w.shape[0], : w.shape[1]] = w
    return out


def _blockdiag4(w):
    out = np.zeros((128, 128), np.float32)
    for g in range(4):
        out[32 * g : 32 * g + 32, 32 * g : 32 * g + 32] = w
    return out


def _build_program():
    nc = bacc.Bacc("TRN2", target_bir_lowering=False, debug=False,
                   num_devices=NCORES)

    # ---- DRAM I/O ----
    CONST = nc.dram_tensor("constpk", [128, CW_TOT], dt.float32, kind="ExternalInput").ap()
    OUTB = nc.dram_tensor("outb", [4, 8], dt.float32, kind="ExternalInput").ap()
    APK = nc.dram_tensor("apk", [NGROUP, 128, 4, 4, 512], dt.float16, kind="ExternalInput").ap()
    ATPK = nc.dram_tensor("atpk", [NGROUP, 128, 4, 4, 512], dt.float16, kind="ExternalInput").ap()
    FPK = nc.dram_tensor("fpk", [NGROUP, 128, 5, 512], dt.float32, kind="ExternalInput").ap()
    OUT = nc.dram_tensor("out", [GPC, 2], dt.float32, kind="ExternalOutput").ap()
    IDXD = nc.dram_tensor("idxd", [NPAIR, 2, 128, 2], dt.int16, kind="Internal").ap()

    with tile.TileContext(nc) as tc, ExitStack() as ctx:
        cpool = ctx.enter_context(tc.tile_pool(name="consts", bufs=1))
        gpool = ctx.enter_context(tc.tile_pool(name="graph", bufs=1))
        apool = ctx.enter_context(tc.tile_pool(name="amat", bufs=2))
        atpoolA = ctx.enter_context(tc.tile_pool(name="atmatA", bufs=2))
        atpoolB = ctx.enter_context(tc.tile_pool(name="atmatB", bufs=2))
        hpool = ctx.enter_context(tc.tile_pool(name="acts", bufs=2))
        hcatp = ctx.enter_context(tc.tile_pool(name="hcat", bufs=4))
        tpool = ctx.enter_context(tc.tile_pool(name="tmp", bufs=2))
        t3pool = ctx.enter_context(tc.tile_pool(name="tmp3", bufs=3))
        kpool = ctx.enter_context(tc.tile_pool(name="keys", bufs=1))
        pzn = ctx.enter_context(tc.tile_pool(name="pzn", bufs=2, space="PSUM"))
        pagg = ctx.enter_context(tc.tile_pool(name="pagg", bufs=4, space="PSUM"))
        ps2 = ctx.enter_context(tc.tile_pool(name="ps2", bufs=2, space="PSUM"))
        # bank budget (8): pzn x3 + pagg x3 + ps2 x2 = 8

        ct = cpool.tile([128, CW_TOT], dt.float32, tag="constpk")
        nc.sync.dma_start(ct[:], CONST)
        outb = cpool.tile([4, 8], dt.float32, tag="outb")
        nc.sync.dma_start(outb[:], OUTB)

        w0 = ct[:, _OFF_W0 : _OFF_W0 + 32]
        bde = [ct[:, _OFF_BDE + 128 * l : _OFF_BDE + 128 * l + 128] for l in range(3)]
        bdn = [ct[:, _OFF_BDN + 128 * l : _OFF_BDN + 128 * l + 128] for l in range(4)]
        bepp = [ct[:, _OFF_BEPP + l : _OFF_BEPP + l + 1] for l in range(4)]
        bnpp = [ct[:, _OFF_BNPP + l : _OFF_BNPP + l + 1] for l in range(4)]
        ident = ct[:, _OFF_IDENT : _OFF_IDENT + 128]
        cw1 = [ct[:, _OFF_CW1 + 16 * l : _OFF_CW1 + 16 * l + 16] for l in range(4)]
        cb1 = ct[:, _OFF_CB1 : _OFF_CB1 + 1]
        cw2 = [ct[:, _OFF_CW2 + 32 * d : _OFF_CW2 + 32 * d + 32] for d in range(5)]
        cb2 = ct[:, _OFF_CB2 : _OFF_CB2 + 1]
        ow = [ct[:, _OFF_OW + 11 * o : _OFF_OW + 11 * o + 11] for o in range(2)]
        ssum = ct[:, _OFF_SSUM : _OFF_SSUM + 4]

        # pair P uses partitions 32P..32P+8 (engine ops need 32-aligned bases)
        keysB = kpool.tile([64, 512], dt.float32, tag="keysB")
        Yout = kpool.tile([128, 8], dt.float32, tag="yout")

        def direction(gi, l, side, st):
            """One message-passing direction for group-slot gi.

            The linear transform is computed with the activations as the
            stationary operand (lhsT = hT chunk, rhs = block-diag weights), so
            the result lands in PSUM directly in contraction-major layout
            [node/edge partition, (chunk, graph, feature) columns] — no PE
            transpose or PSUM->SBUF staging copy is needed.
            """
            hT_in = st["hT"]
            zN = pzn.tile([128, 512], dt.float32, tag="zn")
            if side == "E" and l == 0:
                for c in range(4):
                    for g in range(4):
                        nc.tensor.matmul(
                            zN[:, 128 * c + 32 * g : 128 * c + 32 * g + 32],
                            st["fpk"][:, g, 128 * c : 128 * c + 128], w0,
                            start=True, stop=True)
            else:
                src = hT_in[:] if side == "E" else st["heT"][:]
                bdw = bde[l - 1] if side == "E" else bdn[l]
                for c in range(4):
                    nc.tensor.matmul(zN[:, 128 * c : 128 * c + 128],
                                     src[:, 128 * c : 128 * c + 128], bdw,
                                     start=True, stop=True)
            zhi = t3pool.tile([128, 512], dt.float16, tag="zhi")
            nc.scalar.copy(zhi[:], zN[:])
            zlo = t3pool.tile([128, 512], dt.float16, tag="zlo")
            nc.vector.tensor_tensor(zlo[:], zN[:], zhi[:], ALU.subtract)

            # aggregation, graph-interleaved issue: the four per-graph
            # accumulation chains target distinct 32-col PE tiles so adjacent
            # matmuls stream concurrently; per-graph PSUM accumulation order
            # (zhi c0..c3 then zlo c0..c3) matches the reference kernel.
            Am = st["A"] if side == "E" else st["At"]
            agg = pagg.tile([128, 512], dt.float32, tag="agg")
            n = 0
            for zp in (zhi, zlo):
                for c in range(4):
                    for g in range(4):
                        nc.tensor.matmul(
                            agg[32 * g : 32 * g + 32, :],
                            zp[:, 128 * c + 32 * g : 128 * c + 32 * g + 32],
                            Am[:, g, c, :], start=(n == 0), stop=(n == 7),
                            tile_position=(0, 32 * g))
                    n += 1
            if side == "E":
                ue = tpool.tile([128, 512], dt.float32, tag="ue")
                nc.vector.scalar_tensor_tensor(ue[:], agg[:], bepp[l], st["recip"][:],
                                               ALU.add, ALU.mult)
                heT = hpool.tile([128, 512], dt.float32, tag="heT")
                nc.scalar.activation(heT[:], ue[:], AF.Tanh)
                st["heT"] = heT
            else:
                hT = hcatp.tile([128, 512], dt.float32, tag=f"hT{l}")
                nc.scalar.activation(hT[:], agg[:], AF.Tanh, bias=bnpp[l],
                                     scale=1.0 / HDEG)
                st["hT"] = hT
                st["hcat"][l] = hT
                if l == 3:
                    r = st["krow"]
                    krows = hT[:].rearrange("(a b) f -> a b f", b=32)[:, 0, :]
                    nc.sync.dma_start(keysB[r : r + 4, :], krows)

        all_states = []
        for P in range(NPAIR):
            states = []
            # group-at-a-time: each group's fpk/A DMAs immediately precede its
            # layer-0 E-side so the critical-path transfers get full bandwidth;
            # the later-needed At transfers are issued only afterwards
            for j in range(2):
                G = 2 * P + j
                fpk = gpool.tile([128, 5, 512], dt.float32, tag=f"fpk{j}")
                nc.sync.dma_start(fpk[:], FPK[G])
                A = apool.tile([128, 4, 4, 512], dt.float16, tag=f"A{j}")
                nc.sync.dma_start(A[:], APK[G])
                st = {
                    "G": G,
                    "krow": 32 * P + 4 * j,
                    "fpk": fpk,
                    "A": A[:], "At": None,
                    "hcat": [None] * 4,
                    "hT": None, "heT": None,
                }
                recip = hpool.tile([128, 512], dt.float32, tag="recip")
                nc.vector.tensor_copy(recip[:], fpk[:, 4, :])
                st["recip"] = recip
                states.append(st)
                direction(j, 0, "E", st)
            for j in range(2):
                G = 2 * P + j
                atp = atpoolA if j == 0 else atpoolB
                At = atp.tile([128, 4, 4, 512], dt.float16, tag=f"At{j}")
                nc.sync.dma_start(At[:], ATPK[G])
                states[j]["At"] = At[:]
            for j in range(2):
                direction(j, 0, "N", states[j])
            for l in range(1, 4):
                for side in ("E", "N"):
                    for j in range(2):
                        direction(j, l, side, states[j])

            # ---- per-pair top-k (partitions 32P..32P+8 of the key tiles) ----
            r0 = 32 * P
            kw = keysB   # destructive top-k: keys are not needed afterwards
            idxu = kpool.tile([64, 32], dt.uint32, tag="idxu")
            for r in range(4):
                m8 = kpool.tile([64, 8], dt.float32, tag="m8")
                nc.vector.max(m8[r0 : r0 + 8, :], kw[r0 : r0 + 8, :])
                nc.vector.max_index(idxu[r0 : r0 + 8, 8 * r : 8 * r + 8],
                                    m8[r0 : r0 + 8, :], kw[r0 : r0 + 8, :])
                nc.vector.match_replace(kw[r0 : r0 + 8, :], m8[r0 : r0 + 8, :],
                                        kw[r0 : r0 + 8, :], -1e30)
            idx16 = kpool.tile([64, 32], dt.int16, tag="idx16")
            nc.vector.tensor_copy(idx16[r0 : r0 + 8, :], idxu[r0 : r0 + 8, :])
            # pre-wrap into ap_gather layout: row m becomes 2x-replicated
            # (idx[0], idx[16], idx[1], idx[17], ...) so a plain partition-
            # scatter DMA lands idx i at partition i%16, col i//16
            idx16i = kpool.tile([64, 64], dt.int16, tag="idx16i")
            wsrc = idx16[r0 : r0 + 8, :].rearrange("m (t lo) -> m lo t", lo=16) \
                .unsqueeze(1).broadcast_to([8, 2, 16, 2])
            wdst = idx16i[r0 : r0 + 8, :].rearrange("m (s lo t) -> m s lo t", s=2, t=2)
            nc.vector.tensor_copy(wdst, wsrc)
            for j in range(2):
                nc.sync.dma_start(
                    IDXD[P, j].rearrange("(m p) t -> m (p t)", m=4),
                    idx16i[r0 + 4 * j : r0 + 4 * j + 4, :])

            # ---- pooled gather + conv tower per group of this pair ----
            for j in range(2):
                G = 2 * P + j
                tiles = states[j]["hcat"]
                idxw = tpool.tile([128, 2], dt.int16, tag="idxw")
                nc.sync.dma_start(idxw[:], IDXD[P, j])

                pgs = []
                for l in range(4):
                    pg = tpool.tile([128, 32], dt.float32, tag=f"pg{l}")
                    nc.gpsimd.ap_gather(pg[:], tiles[l][:].unsqueeze(2), idxw[:],
                                        channels=128, num_elems=512, d=1, num_idxs=32)
                    pgs.append(pg)

                y1 = ps2.tile([128, 30], dt.float32, tag="small")
                for g in range(4):
                    for l in range(4):
                        nc.tensor.matmul(y1[32 * g : 32 * g + 16, :],
                                         cw1[l][32 * g : 32 * g + 32, :],
                                         pgs[l][32 * g : 32 * g + 32, 0:30],
                                         start=(l == 0), stop=(l == 3),
                                         tile_position=(32 * g, 32 * g))
                y1r = tpool.tile([128, 30], dt.float32, tag="y1r")
                nc.scalar.activation(y1r[:], y1[:], AF.Relu, bias=cb1)
                y1p = tpool.tile([128, 15], dt.float32, tag="y1p")
                nc.vector.tensor_tensor(
                    y1p[:], y1r[:].rearrange("p (t two) -> p t two", two=2)[:, :, 0],
                    y1r[:].rearrange("p (t two) -> p t two", two=2)[:, :, 1], ALU.max)

                y2 = ps2.tile([128, 11], dt.float32, tag="small")
                for g in range(4):
                    for d in range(5):
                        nc.tensor.matmul(y2[32 * g : 32 * g + 32, :],
                                         cw2[d][32 * g : 32 * g + 32, :],
                                         y1p[32 * g : 32 * g + 32, d : d + 11],
                                         start=(d == 0), stop=(d == 4),
                                         tile_position=(32 * g, 32 * g))
                y2r = tpool.tile([128, 11], dt.float32, tag="y2r")
                nc.scalar.activation(y2r[:], y2[:], AF.Relu, bias=cb2)
                for o in range(2):
                    t_o = tpool.tile([128, 11], dt.float32, tag="t_o")
                    nc.vector.tensor_tensor(t_o[:], y2r[:], ow[o], ALU.mult)
                    nc.vector.tensor_reduce(Yout[:, 2 * G + o : 2 * G + o + 1],
                                            t_o[:], AX.X, ALU.add)

        # ---------- final dense + relu + output ----------
        pout = ps2.tile([4, 8], dt.float32, tag="small")
        nc.tensor.matmul(pout[:], ssum, Yout[:], start=True, stop=True)
        ob = kpool.tile([4, 8], dt.float32, tag="ob")
        nc.vector.tensor_tensor(ob[:], pout[:], outb[:], ALU.add)
        orl = kpool.tile([4, 8], dt.float32, tag="orl")
        nc.scalar.activation(orl[:], ob[:], AF.Relu)
        nc.sync.dma_start(OUT.rearrange("(G g) o -> g G o", g=4), orl[:])

    nc.compile()
    return nc


def _make_consts(inputs):
    ws = [inputs[f"w{i}"].astype(np.float32) for i in range(8)]
    bs = [inputs[f"b{i}"].astype(np.float32) for i in range(8)]
    wE = [ws[0], _pad32(ws[2]), _pad32(ws[4]), _pad32(ws[6])]
    wN = [_pad32(ws[1]), _pad32(ws[3]), _pad32(ws[5]), _pad32(ws[7])]
    bE = [bs[0], bs[2], bs[4], np.pad(bs[6], (0, 31))]
    bN = [bs[1], bs[3], bs[5], np.pad(bs[7], (0, 31))]

    cpk = np.zeros((128, CW_TOT), np.float32)
    for l in range(1, 4):
        cpk[:, _OFF_BDE + 128 * (l - 1) : _OFF_BDE + 128 * l] = _blockdiag4(wE[l])
    for l in range(4):
        cpk[:, _OFF_BDN + 128 * l : _OFF_BDN + 128 * (l + 1)] = _blockdiag4(wN[l])
    cpk[:, _OFF_IDENT : _OFF_IDENT + 128] = np.eye(128, dtype=np.float32)
    cpk[:, _OFF_W0 : _OFF_W0 + 32] = ws[0]
    for l in range(4):
        cpk[:, _OFF_BEPP + l] = np.tile(bE[l], 4)
        cpk[:, _OFF_BNPP + l] = np.tile(bN[l], 4) / HDEG

    c1w = inputs["conv1_w"].astype(np.float32).reshape(C1, 97)    # [16, 97]
    for l in range(4):
        blk = np.zeros((32, 16), np.float32)
        if l < 3:
            blk = c1w[:, 32 * l : 32 * l + 32].T
        else:
            blk[0, :] = c1w[:, 96]
        for g in range(4):
            cpk[32 * g : 32 * g + 32, _OFF_CW1 + 16 * l : _OFF_CW1 + 16 * (l + 1)] = blk
    for g in range(4):
        cpk[32 * g : 32 * g + 16, _OFF_CB1] = inputs["conv1_b"]
    c2w = inputs["conv2_w"].astype(np.float32)                    # [32, 16, 5]
    for d in range(5):
        for g in range(4):
            cpk[32 * g : 32 * g + 16, _OFF_CW2 + 32 * d : _OFF_CW2 + 32 * (d + 1)] = c2w[:, :, d].T
    for g in range(4):
        cpk[32 * g : 32 * g + 32, _OFF_CB2] = inputs["conv2_b"]
    oww = inputs["out_w"].astype(np.float32)                      # [352, 2]
    for o in range(2):
        for g in range(4):
            cpk[32 * g : 32 * g + 32, _OFF_OW + 11 * o : _OFF_OW + 11 * (o + 1)] = \
                oww[:, o].reshape(C2, 11)
    for j in range(4):
        cpk[32 * j : 32 * j + 32, _OFF_SSUM + j] = 1.0

    outb = np.tile(inputs["out_b"].astype(np.float32), (4, 4))    # [4, 8]
    return {"constpk": cpk, "outb": outb}


def _build_incidence(inputs):
    """Host-side A / At / hyperedge-size construction (exact small ints)."""
    einc_g = np.asarray(inputs["inc_edge"]).reshape(B, NPER, DEG).astype(np.int64)
    base = (np.arange(B, dtype=np.int64) * EPER)[:, None, None]
    loc = einc_g - base                                           # [B, 512, 32] local
    A_all = np.empty((B, 128, 4, EPER), np.float16)
    At_all = np.empty((B, 128, 4, NPER), np.float16)
    hs_all = np.empty((B, EPER), np.float32)
    rowbase = (np.arange(NPER, dtype=np.int64) * EPER)[:, None]
    for g in range(B):
        flat = (rowbase + loc[g]).ravel()
        cnt = np.bincount(flat, minlength=NPER * EPER).reshape(NPER, EPER)
        c16 = cnt.astype(np.float16)                              # [n, e]
        A_all[g] = c16.reshape(4, 128, EPER).transpose(1, 0, 2)
        At_all[g] = np.ascontiguousarray(c16.T).reshape(4, 128, NPER).transpose(1, 0, 2)
        hs_all[g] = cnt.sum(axis=0).astype(np.float32) + 1.0
    return A_all, At_all, hs_all


def make_in_maps(inputs):
    consts = _make_consts(inputs)
    nf = np.asarray(inputs["node_feat"]).astype(np.float32).reshape(B, NPER, F)
    nfT = nf.transpose(0, 2, 1)                                   # [B, 128f, 512n]
    A_all, At_all, hs_all = _build_incidence(inputs)
    in_maps = []
    for c in range(NCORES):
        m = dict(consts)
        sl = slice(c * GPC, (c + 1) * GPC)
        # apk/atpk: [NGROUP, 128, 4g, 4c, 512]
        m["apk"] = np.ascontiguousarray(
            A_all[sl].reshape(NGROUP, 4, 128, 4, EPER).transpose(0, 2, 1, 3, 4))
        m["atpk"] = np.ascontiguousarray(
            At_all[sl].reshape(NGROUP, 4, 128, 4, NPER).transpose(0, 2, 1, 3, 4))
        # fpk: [NGROUP, 128, 5, 512] = 4 transposed-feature graphs + hsize row
        fpk = np.empty((NGROUP, 128, 5, 512), np.float32)
        nfT_c = nfT[sl].reshape(NGROUP, 4, 128, NPER)
        for G in range(NGROUP):
            for g in range(4):
                fpk[G, :, g, :] = nfT_c[G, g]
        hs_core = np.divide(np.float32(1.0), hs_all[sl],
                            dtype=np.float32).reshape(NGROUP, 4, EPER)
        fpk[:, :, 4, :] = np.repeat(hs_core[:, :, None, :], 32, axis=2).reshape(
            NGROUP, 128, EPER)
        m["fpk"] = np.ascontiguousarray(fpk)
        in_maps.append(m)
    return in_maps


def get_program():
    if "nc" not in _CACHE:
        _CACHE["nc"] = _build_program()
    return _CACHE["nc"]


def kernel(**inputs):
    nc = get_program()
    in_maps = make_in_maps(inputs)
    res = run_bass_kernel_spmd(nc, in_maps, core_ids=list(range(NCORES)))
    out = np.concatenate([res.results[c]["out"] for c in range(NCORES)], axis=0)
    return out.astype(np.float32)
